# revision 1
# baseline (speedup 1.0000x reference)
"""Swin-style windowed attention (B_=2048 windows, N=49 tokens, C=512, 16 heads)
on 8 Trainium2 NeuronCores, data-parallel over windows (256 windows/core).

Layout strategy (per core):
  - host pre-transposes x -> xT [C, 12560] bf16 (feature-major, 16-token
    zero pad), pre-transposes weights, folds the 1/sqrt(hd) scale into the q
    columns, and precomputes exp(relative-position-bias) tiles.
  - q,k computed feature-major (head h lands at partition 32*(h%4) of tile
    h//4), v computed token-major with the two windows of a pair at partition
    slots {0, 64} so window rows are 32-aligned for tile_position addressing.
  - scores^T via K=32 row-packed matmuls; matmuls with different tile_position
    rows must target different PSUM banks (HW constraint), so scores use one
    PSUM tile per row group (4 heads each) and AV uses one PSUM tile per
    window slot.
  - rel-pos bias applied as multiplicative exp(bias) on GPSIMD after the ACT
    exp; AV + softmax denominator fused in one matmul per (window, head) via a
    v tile with interleaved ones columns (N=33); normalization = DVE
    reciprocal + free-broadcast multiply; attention output transposed back to
    feature-major on the PE for the final projection.
"""

import os
import sys

import numpy as np
import ml_dtypes

if "/opt/trn_rl_repo" not in sys.path:
    sys.path.insert(0, "/opt/trn_rl_repo")

P = 49          # tokens per window
NH = 16         # heads
HD = 32         # head dim
C = 512         # model dim
NCORES = 8
B_TOTAL = 2048
B_CORE = B_TOTAL // NCORES        # 256 windows per core
TOK = B_CORE * P                  # 12544 tokens per core
OCT = 32                          # octets (8 windows) per core
OCT_TOK = 8 * P                   # 392 tokens per octet
OCT_W = OCT_TOK + 16              # octet tile width incl 16-token overlap
TOK_PAD = TOK + 16
BF16 = ml_dtypes.bfloat16


def _build(nc, has_bqk, has_bv, has_bp, n_oct=OCT):
    import concourse.bass as bass
    import concourse.mybir as mybir
    from concourse.tile import TileContext
    from concourse.masks import make_identity

    F32 = mybir.dt.float32
    BF = mybir.dt.bfloat16
    Exp = mybir.ActivationFunctionType.Exp

    xT = nc.dram_tensor("xT", [C, TOK_PAD], BF, kind="ExternalInput")
    wqk = nc.dram_tensor("wqk", [C, 2 * C], BF, kind="ExternalInput")
    wv = nc.dram_tensor("wv", [C, C], BF, kind="ExternalInput")
    wp = nc.dram_tensor("wp", [C, C], BF, kind="ExternalInput")
    eb = nc.dram_tensor("eb", [128, 4, 4, P], F32, kind="ExternalInput")
    bqk = bv = bp = None
    if has_bqk:
        bqk = nc.dram_tensor("bqk", [1, 2 * C], BF, kind="ExternalInput")
    if has_bv:
        bv = nc.dram_tensor("bv", [1, C], BF, kind="ExternalInput")
    if has_bp:
        bp = nc.dram_tensor("bp", [1, C], BF, kind="ExternalInput")
    out = nc.dram_tensor("out", [TOK, C], BF, kind="ExternalOutput")

    def bcast_last(ap, n):
        return bass.AP(ap.tensor, ap.offset, [*ap.ap, [0, n]])

    with TileContext(nc) as tc:
        with (
            tc.tile_pool(name="singles", bufs=1) as singles,
            tc.tile_pool(name="xt", bufs=2) as xt_pool,
            tc.tile_pool(name="qk", bufs=2) as qk_pool,
            tc.tile_pool(name="vsb", bufs=8) as v_pool,
            tc.tile_pool(name="se", bufs=3) as se_pool,
            tc.tile_pool(name="zr", bufs=4) as zr_pool,
            tc.tile_pool(name="attn", bufs=6) as attn_pool,
            tc.tile_pool(name="att", bufs=4) as atT_pool,
            tc.tile_pool(name="osb", bufs=4) as out_pool,
            tc.tile_pool(name="ps_big", bufs=2, space="PSUM") as ps_big,
            tc.tile_pool(name="ps_st", bufs=1, space="PSUM") as ps_st,
            tc.tile_pool(name="ps_av", bufs=1, space="PSUM") as ps_av,
        ):
            # --- constants / weights ---
            wqk_sb = []
            wv_sb = []
            wp_sb = []
            for ci in range(4):
                wqk_t = singles.tile([128, 2 * C], BF, name=f"wqk{ci}")
                nc.sync.dma_start(out=wqk_t, in_=wqk[128 * ci:128 * (ci + 1), :])
                wqk_sb.append(wqk_t)
                wv_t = singles.tile([128, C], BF, name=f"wv{ci}")
                nc.sync.dma_start(out=wv_t, in_=wv[128 * ci:128 * (ci + 1), :])
                wv_sb.append(wv_t)
                wp_t = singles.tile([128, C], BF, name=f"wp{ci}")
                nc.sync.dma_start(out=wp_t, in_=wp[128 * ci:128 * (ci + 1), :])
                wp_sb.append(wp_t)
            eb_sb = singles.tile([128, 4, 4, P], F32, name="ebsb")
            nc.sync.dma_start(out=eb_sb, in_=eb[:, :, :, :])
            ident = singles.tile([128, 128], BF, name="ident")
            make_identity(nc, ident)
            bqk_sb = bv_sb = bp_sb = ones_row = None
            if has_bqk or has_bv or has_bp:
                ones_row = singles.tile([1, OCT_W], BF, name="onesrow")
                nc.vector.memset(ones_row, 1.0)
            if has_bqk:
                bqk_sb = singles.tile([1, 2 * C], BF, name="bqksb")
                nc.sync.dma_start(out=bqk_sb, in_=bqk[:, :])
            if has_bv:
                bv_sb = singles.tile([1, C], BF, name="bvsb")
                nc.sync.dma_start(out=bv_sb, in_=bv[:, :])
            if has_bp:
                bp_sb = singles.tile([1, C], BF, name="bpsb")
                nc.sync.dma_start(out=bp_sb, in_=bp[:, :])

            # --- main loop ---
            for o in range(n_oct):
                t0 = o * OCT_TOK
                xts = []
                xt2s = []
                for ci in range(4):
                    xt_t = xt_pool.tile([128, OCT_W], BF, name=f"xt{o}_{ci}",
                                        tag=f"xt{ci}")
                    nc.sync.dma_start(
                        out=xt_t,
                        in_=xT[128 * ci:128 * (ci + 1), t0:t0 + OCT_W])
                    xts.append(xt_t)
                    # slot-expanded copy for the v projection: per pair the
                    # two windows at 64-aligned positions ({0..63, 49..112})
                    xt2_t = xt_pool.tile([128, 4, 128], BF, name=f"xt2{o}_{ci}",
                                         tag=f"xt2{ci}")
                    xr = xT[128 * ci:128 * (ci + 1), :]
                    src_ap = bass.AP(xr.tensor, xr.offset + t0,
                                     [xr.ap[0], [98, 4], [P, 2], [1, 64]])
                    nc.sync.dma_start(out=xt2_t, in_=src_ap)
                    xt2s.append(xt2_t)

                # q,k feature-major: qks[0:4]=q tiles, qks[4:8]=k tiles
                qks = []
                for ft in range(8):
                    ps = ps_big.tile([128, OCT_W], F32, name=f"qkp{o}_{ft}",
                                     tag="big")
                    for ci in range(4):
                        nc.tensor.matmul(ps,
                                         wqk_sb[ci][:, 128 * ft:128 * (ft + 1)],
                                         xts[ci], start=(ci == 0),
                                         stop=(ci == 3 and not has_bqk))
                    if has_bqk:
                        nc.tensor.matmul(ps, bqk_sb[:, 128 * ft:128 * (ft + 1)],
                                         ones_row, start=False, stop=True)
                    sb = qk_pool.tile([128, OCT_W], BF, name=f"qk{o}_{ft}",
                                      tag=f"qk{ft}")
                    if ft < 4:
                        nc.scalar.copy(sb, ps)
                    else:
                        nc.vector.tensor_copy(sb, ps)
                    qks.append(sb)

                # phases 2+3 merged per pair: v chain is short
                # (matmuls -> one copy) so pairs pipeline through
                # the big pool without blocking attention
                v_sbs = []
                attn_sbs = []
                for p in range(4):
                    pt0 = 98 * p
                    vps = ps_big.tile([128, C], F32, name=f"vp{o}_{p}", tag="big")
                    for ci in range(4):
                        # both windows at slot-aligned output partitions 0/64
                        # in a single M=128 matmul (slot-expanded x copy)
                        nc.tensor.matmul(
                            vps, xt2s[ci][:, p, :],
                            wv_sb[ci], start=(ci == 0),
                            stop=(ci == 3 and not has_bv))
                    if has_bv:
                        nc.tensor.matmul(vps, ones_row[:, 0:128], bv_sb,
                                         start=False, stop=True)
                    v_sb = v_pool.tile([128, NH, 33], BF, name=f"v{o}_{p}", tag="v")
                    nc.scalar.copy(
                        v_sb[:, :, 0:32],
                        vps.rearrange("q (h d) -> q h d", h=NH))
                    nc.gpsimd.memset(v_sb[:, :, 32], 1.0)
                    v_sbs.append(v_sb)


                    pt0 = 98 * p
                    v_sb = v_sbs[p]
                    # scores^T: one PSUM tile per row group j (heads j, j+4,
                    # j+8, j+12); both window slots share the tile (same
                    # tile_position row, cols 0/64)
                    ses = []
                    for j in range(4):
                        stp = ps_st.tile([128, 4, P], F32,
                                         name=f"st{o}_{p}_{j}", tag=f"st{j}")
                        r = 32 * j
                        for i in range(4):
                            h = 4 * i + j
                            qq = qks[h // 4]
                            kk = qks[4 + h // 4]
                            for wi in range(2):
                                s = 64 * wi
                                wtok = pt0 + P * wi
                                nc.tensor.matmul(
                                    stp[s:s + 64, i, :],
                                    kk[r:r + 32, wtok:wtok + 64],
                                    qq[r:r + 32, wtok:wtok + P],
                                    start=True, stop=True,
                                    tile_position=(r, s))
                        se = se_pool.tile([128, 4, P], BF,
                                          name=f"se{o}_{p}_{j}", tag=f"se{j}")
                        nc.scalar.activation(out=se, in_=stp, func=Exp)
                        nc.gpsimd.tensor_mul(se, se, eb_sb[:, j, :, :])
                        ses.append(se)

                    attn_sb = attn_pool.tile([128, NH, HD], BF,
                                             name=f"attn{o}_{p}", tag="attn")
                    nc.gpsimd.memset(attn_sb, 0.0)   # define gap rows
                    # head h = 4a + c; AV pass G covers c in {2G, 2G+1} so it
                    # only needs se tiles j=2G, 2G+1 (starts before j=2G+2 is
                    # ready). attn_sb viewed [q, a, c, d] for scattered writes.
                    attn_v = attn_sb.rearrange("q (a c) d -> q a c d", a=4)
                    for G in range(2):
                        avs = []
                        for wi in range(2):
                            s = 64 * wi
                            av = ps_av.tile([128, 4, 2, 33], F32,
                                            name=f"av{o}_{p}_{G}_{wi}",
                                            tag=f"av{wi}")
                            for a in range(4):
                                for cc in range(2):
                                    c = 2 * G + cc
                                    h = 4 * a + c
                                    nc.tensor.matmul(
                                        av[s:s + P, a, cc, :],
                                        ses[c][s:s + P, a, :],
                                        v_sb[s:s + P, h, :],
                                        start=True, stop=True,
                                        tile_position=(s, s))
                            avs.append(av)
                        zr = zr_pool.tile([128, 4, 2], F32,
                                          name=f"zr{o}_{p}_{G}", tag="zr")
                        for wi in range(2):
                            s = 64 * wi
                            av = avs[wi]
                            nc.vector.reciprocal(zr[s:s + P, :, :],
                                                 av[s:s + P, :, :, 32])
                            nc.vector.tensor_mul(
                                attn_v[s:s + P, :, 2 * G:2 * G + 2, :],
                                av[s:s + P, :, :, 0:32],
                                bcast_last(zr[s:s + P, :, :], HD))
                    attn_sbs.append(attn_sb)

                # phase 4: transpose + projection per pair
                for p in range(4):
                    gt0 = t0 + 98 * p
                    atp = ps_big.tile([128, 4, 128], BF, name=f"atp{o}_{p}",
                                      tag="big")
                    attn_flat = attn_sbs[p].rearrange("q h d -> q (h d)")
                    for ci in range(4):
                        nc.tensor.transpose(atp[:, ci, :],
                                            attn_flat[:, 128 * ci:128 * (ci + 1)],
                                            ident)
                    atT_sb = atT_pool.tile([128, 4, 128], BF, name=f"atT{o}_{p}",
                                           tag="atT")
                    nc.vector.tensor_copy(atT_sb, atp)

                    pso = ps_big.tile([128, C], F32, name=f"po{o}_{p}", tag="big")
                    for ci in range(4):
                        nc.tensor.matmul(pso, atT_sb[:, ci, :], wp_sb[ci],
                                         start=(ci == 0),
                                         stop=(ci == 3 and not has_bp))
                    if has_bp:
                        nc.tensor.matmul(pso, ones_row[:, 0:128], bp_sb,
                                         start=False, stop=True)
                    osb = out_pool.tile([128, C], BF, name=f"o{o}_{p}", tag="osb")
                    nc.scalar.copy(osb, pso)
                    nc.sync.dma_start(out=out[gt0:gt0 + P, :], in_=osb[0:P, :])
                    nc.sync.dma_start(out=out[gt0 + P:gt0 + 2 * P, :],
                                      in_=osb[64:64 + P, :])
    return nc


def _host_prep(x, qkv_w, qkv_b, proj_w, proj_b, rpb_table, rel_index):
    scale = HD ** -0.5
    # weights: qkv feature order is (3, NH, HD) -> q=0:512, k=512:1024, v=1024:1536
    wq = qkv_w[0:C, :] * scale          # fold scale into q
    wk = qkv_w[C:2 * C, :]
    wv = qkv_w[2 * C:3 * C, :]
    wqk = np.concatenate([wq.T, wk.T], axis=1).astype(BF16)     # [C, 2C]
    wv_t = np.ascontiguousarray(wv.T).astype(BF16)              # [C, C]
    wp_t = np.ascontiguousarray(proj_w.T).astype(BF16)          # [C, C]

    bias = rpb_table[rel_index]                  # [n, m, NH], attn[h,n,m] += bias[n,m,h]
    biasT = np.transpose(bias, (2, 1, 0))        # [h, m, n]
    ebias = np.exp(biasT.astype(np.float64)).astype(np.float32)
    ebp = np.ones((128, NH, P), np.float32)
    ebp[0:P] = np.transpose(ebias, (1, 0, 2))    # rows 0:49  (window slot 0)
    ebp[64:64 + P] = ebp[0:P]                    # rows 64:113 (window slot 1)
    # regroup heads: [128, j, i, P] with head 4*i + j at [:, j, i, :]
    eb = np.ascontiguousarray(
        ebp.reshape(128, 4, 4, P).transpose(0, 2, 1, 3))

    bq = qkv_b[0:C] * scale
    bk = qkv_b[C:2 * C]
    bv_ = qkv_b[2 * C:3 * C]
    bqk = np.concatenate([bq, bk])[None, :].astype(BF16)
    bv = bv_[None, :].astype(BF16)
    bp = proj_b[None, :].astype(BF16)

    has_bqk = bool(np.any(bqk != 0))
    has_bv = bool(np.any(bv != 0))
    has_bp = bool(np.any(bp != 0))

    in_maps = []
    for c in range(NCORES):
        xc = x[c * B_CORE:(c + 1) * B_CORE].reshape(TOK, C)
        xTc = np.zeros((C, TOK_PAD), BF16)
        xTc[:, :TOK] = xc.T.astype(BF16)
        m = {"xT": xTc, "wqk": wqk, "wv": wv_t, "wp": wp_t, "eb": eb}
        if has_bqk:
            m["bqk"] = bqk
        if has_bv:
            m["bv"] = bv
        if has_bp:
            m["bp"] = bp
        in_maps.append(m)
    return in_maps, has_bqk, has_bv, has_bp


def kernel(x, qkv_w, qkv_b, proj_w, proj_b, rpb_table, rel_index):
    from concourse import bacc
    from concourse.bass_utils import run_bass_kernel_spmd

    in_maps, has_bqk, has_bv, has_bp = _host_prep(
        np.asarray(x, np.float32), np.asarray(qkv_w, np.float32),
        np.asarray(qkv_b, np.float32), np.asarray(proj_w, np.float32),
        np.asarray(proj_b, np.float32), np.asarray(rpb_table, np.float32),
        np.asarray(rel_index))

    nc = bacc.Bacc()
    _build(nc, has_bqk, has_bv, has_bp)
    nc.finalize()

    trace = os.environ.get("BASS_KERNEL_TRACE", "") == "1"
    res = run_bass_kernel_spmd(nc, in_maps, core_ids=list(range(NCORES)),
                               trace=trace)
    if trace and res.exec_time_ns is not None:
        print(f"HW exec time: {res.exec_time_ns} ns", flush=True)

    outs = [r["out"].astype(np.float32).reshape(B_CORE, P, C)
            for r in res.results]
    return np.concatenate(outs, axis=0)

